# revision 3
# baseline (speedup 1.0000x reference)
"""Block-causal (anti-causal: key-block >= query-block) multi-head attention
for Trainium2, run SPMD on 8 NeuronCores.

Problem (hardcoded): B=2, T=8, N=256 (L=2048), D=768, H=12, HD=64.
reference:
    qkv = x @ qkv_w.T + qkv_b ; split into q,k,v heads
    s   = (q @ k.T) / 8 ; mask: query in block ti attends keys in blocks tj >= ti
    p   = softmax(s) ; y = p @ v ; out = y @ proj_w.T + proj_b

Sharding: data-parallel over B (2) x tensor-parallel over heads (4 groups of
3 heads) = 8 cores. Per core:
  - Q^T,K^T  = Wsel @ x^T   (transposed layout, bias folded into the ACT-engine
               PSUM->SBUF copy as a per-partition bias)
  - V natural layout directly: Vn[tok, v] = x @ Wv (+ ones-free bias matmul)
  - S^T      = K^T-chunks vs Q^T (keys on partitions)
  - P~       = exp(0.125 * S^T)  on ACT
  - AV: weights [V_h | 64 ones-columns] -> psum rows 0:64 = U^T (unnormalized
        out), rows 64:128 = softmax denominator broadcast across 64 partitions
  - normalize: reciprocal + tensor_tensor multiply on DVE
  - Z^T     += Wproj-slice @ O^T  per query-quartet, stored bf16
Host sums the 4 head-group partials per batch and adds proj_b.
"""

import functools

import ml_dtypes
import numpy as np

import concourse.bass as bass
import concourse.bacc as bacc_mod
import concourse.mybir as mybir
import concourse.tile as tile
from concourse.bass import ts

F32 = mybir.dt.float32
BF16 = mybir.dt.bfloat16

B, T, N, D = 2, 8, 256, 768
H, HD = 12, 64
L = T * N          # 2048
HPC = 3            # heads per core
NKC = L // 128     # 16 key chunks of 128
NDC = D // 128     # 6 contraction chunks
NNT = L // 512     # 4 tiles of 512 along L
SCALE = 1.0 / 8.0
LA = 2             # S-tile lookahead before AV consumption


def build_nc():
    nc = bacc_mod.Bacc()

    xT_d = nc.declare_dram_parameter("xT", [D, L], BF16, isOutput=False)
    wqkT_d = nc.declare_dram_parameter("wqkT", [D, 384], BF16, isOutput=False)
    wvT_d = nc.declare_dram_parameter("wvT", [D, 192], BF16, isOutput=False)
    bqkP_d = nc.declare_dram_parameter("bqkP", [128, 3], F32, isOutput=False)
    bv1_d = nc.declare_dram_parameter("bv1", [1, 192], BF16, isOutput=False)
    ones_d = nc.declare_dram_parameter("ones", [1, 512], BF16, isOutput=False)
    wprojT_d = nc.declare_dram_parameter("wprojT", [128, 1536], BF16, isOutput=False)
    zT_d = nc.declare_dram_parameter("zT", [D, L], BF16, isOutput=True)

    with tile.TileContext(nc) as tc:
        with (
            tc.tile_pool(name="persist", bufs=1) as pp,
            tc.tile_pool(name="ptile", bufs=6) as ptp,
            tc.tile_pool(name="invp", bufs=2) as invp,
            tc.tile_pool(name="zbuf", bufs=3) as zbp,
            tc.tile_pool(name="psQ", bufs=2, space="PSUM") as psQ,
            tc.tile_pool(name="psS", bufs=3, space="PSUM") as psS,
            tc.tile_pool(name="psAV", bufs=3, space="PSUM") as psAV,
        ):
            # ---- persistent SBUF tensors ----
            xT = pp.tile([128, NDC, L], BF16, tag="xT")
            wqkT = pp.tile([128, NDC, 384], BF16, tag="wqkT")
            wvT = pp.tile([128, NDC, 192], BF16, tag="wvT")
            wprojT = pp.tile([128, 1536], BF16, tag="wprojT")
            bqkP = pp.tile([128, 3], F32, tag="bqkP")
            bv1 = pp.tile([1, 192], BF16, tag="bv1")
            ones = pp.tile([1, 512], BF16, tag="ones")
            qt = pp.tile([128, L], BF16, tag="qt")      # [q_h0 | q_h1]
            kt = pp.tile([128, L], BF16, tag="kt")      # [k_h0 | k_h1]
            qk2 = pp.tile([128, L], BF16, tag="qk2")    # [q_h2 | k_h2]
            kt2 = pp.tile([64, L], BF16, tag="kt2")     # k_h2 on partitions 0:64
            vn = [
                pp.tile([128, NKC, 128], BF16, tag=f"vn{h}", name=f"vn{h}")
                for h in range(HPC)
            ]
            otp = pp.tile([128, L], BF16, tag="otp")    # [o_h0 | o_h1] transposed
            ots = pp.tile([64, L], BF16, tag="ots")     # o_h2 transposed

            # ---- input DMAs ----
            for dc in range(NDC):
                nc.sync.dma_start(out=wqkT[:, dc, :], in_=wqkT_d[ts(dc, 128), :])
                nc.sync.dma_start(out=wvT[:, dc, :], in_=wvT_d[ts(dc, 128), :])
            nc.sync.dma_start(out=wprojT[:], in_=wprojT_d[:, :])
            nc.sync.dma_start(out=bqkP[:], in_=bqkP_d[:, :])
            nc.sync.dma_start(out=bv1[:], in_=bv1_d[:, :])
            nc.sync.dma_start(out=ones[:], in_=ones_d[:, :])
            for nt in range(NNT):
                for dc in range(NDC):
                    nc.sync.dma_start(
                        out=xT[:, dc, ts(nt, 512)],
                        in_=xT_d[ts(dc, 128), ts(nt, 512)],
                    )

            # ones columns of the AV weights (denominator broadcast)
            for h in range(HPC):
                nc.vector.memset(vn[h][:, :, 64:128], 1.0)
            # Pre-warm the exp activation table.
            warm = zbp.tile([128, 32], F32, tag="warm")
            nc.vector.memset(warm[:], 0.0)
            nc.scalar.activation(warm[:], warm[:], mybir.ActivationFunctionType.Exp)

            # ---- helpers ----
            def qk_group(mc, nt, dst):
                """One QKV-transposed m-chunk for one 512-token tile."""
                ps = psQ.tile([128, 512], F32, tag="q")
                for dc in range(NDC):
                    nc.tensor.matmul(
                        ps[:],
                        wqkT[:, dc, ts(mc, 128)],
                        xT[:, dc, ts(nt, 512)],
                        start=(dc == 0),
                        stop=(dc == NDC - 1),
                    )
                nc.scalar.activation(
                    dst[:, ts(nt, 512)],
                    ps[:],
                    mybir.ActivationFunctionType.Identity,
                    bias=bqkP[:, mc : mc + 1],
                )

            def vnat_chunk(c):
                """V in natural layout (+bias) for one 128-token chunk."""
                pv = psQ.tile([128, 512], F32, tag="q")
                for dc in range(NDC):
                    nc.tensor.matmul(
                        pv[:, 0:192],
                        xT[:, dc, ts(c, 128)],
                        wvT[:, dc, :],
                        start=(dc == 0),
                        stop=False,
                    )
                nc.tensor.matmul(
                    pv[:, 0:192], ones[0:1, 0:128], bv1[0:1, :],
                    start=False, stop=True,
                )
                for h in range(HPC):
                    nc.vector.tensor_copy(
                        vn[h][:, c, 0:64], pv[:, ts(h, 64)]
                    )

            qt_src = [qt[0:64, :], qt[64:128, :], qk2[0:64, :]]
            kt_src = [kt[0:64, :], kt[64:128, :], kt2[0:64, :]]
            ot_dst = [otp[0:64, :], otp[64:128, :], ots[0:64, :]]

            def proj_mc(qq, mc):
                pj = psQ.tile([128, 512], F32, tag="q")
                nc.tensor.matmul(
                    pj[:],
                    wprojT[:, ts(mc, 128)],
                    otp[:, ts(qq, 512)],
                    start=True,
                    stop=False,
                )
                nc.tensor.matmul(
                    pj[:],
                    wprojT[0:64, 768 + mc * 128 : 768 + (mc + 1) * 128],
                    ots[0:64, ts(qq, 512)],
                    start=False,
                    stop=True,
                )
                zb = zbp.tile([128, 512], BF16, tag="zb")
                nc.vector.tensor_copy(zb[:], pj[:])
                nc.sync.dma_start(out=zT_d[ts(mc, 128), ts(qq, 512)], in_=zb[:])

            # ---- phase A: K for all tokens, Q for quartet 0 ----
            for nt in range(NNT):
                qk_group(1, nt, kt)
            qk_group(0, 0, qt)

            # ---- attention quartets ----
            for qq in range(4):
                qb0 = 2 * qq
                q_lo = qb0 * 256
                if qq >= 1:
                    qk_group(0, qq, qt)
                for h in range(HPC):
                    kcs = list(range(4 * qq, NKC))
                    av = psAV.tile([128, 512], F32, tag="av")
                    pts = {}

                    # extras: independent work interleaved into this head's
                    # stream (one item per S-step) to keep PE dense while the
                    # ACT engine catches up on exps
                    extras = []
                    if qq == 0 and h == 0:
                        extras = [
                            (lambda c=c: vnat_chunk(c)) for c in range(NKC)
                        ]
                    elif qq == 0 and h == 1:
                        extras = [
                            (lambda nt=nt: qk_group(2, nt, qk2))
                            for nt in range(NNT)
                        ] + [
                            lambda: nc.gpsimd.dma_start(
                                out=kt2[0:64, :], in_=qk2[64:128, :]
                            )
                        ]
                    elif qq >= 1 and h == 0:
                        extras = [
                            (lambda mc=mc: proj_mc(qq - 1, mc))
                            for mc in range(NDC)
                        ]

                    def emit_av(kc):
                        pt, seg = pts.pop(kc)
                        for qb in (qb0, qb0 + 1):
                            if kc < 2 * qb:
                                continue
                            qrel = (qb - qb0) * 256 if seg == 512 else 0
                            nc.tensor.matmul(
                                av[:, ts(qb - qb0, 256)],
                                vn[h][:, kc, :],
                                pt[:, qrel : qrel + 256],
                                start=(kc == 4 * qq and qb == qb0),
                                stop=(kc == NKC - 1),
                                skip_group_check=True,
                            )

                    for i, kc in enumerate(kcs):
                        seg = 256 if kc < 4 * qq + 2 else 512
                        st = psS.tile([128, 512], F32, tag="s")
                        nc.tensor.matmul(
                            st[:, 0:seg],
                            kt_src[h][:, ts(kc, 128)],
                            qt_src[h][:, q_lo : q_lo + seg],
                            start=True,
                            stop=True,
                        )
                        pt = ptp.tile([128, 512], BF16, tag="pt")
                        nc.scalar.activation(
                            pt[:, 0:seg],
                            st[:, 0:seg],
                            mybir.ActivationFunctionType.Exp,
                            scale=SCALE,
                        )
                        pts[kc] = (pt, seg)
                        if extras:
                            extras.pop(0)()
                        if i >= LA:
                            emit_av(kcs[i - LA])
                    while extras:
                        extras.pop(0)()
                    for kc in kcs[-LA:]:
                        emit_av(kc)

                    # normalize: rows 64:128 of av hold the denominator
                    inv = invp.tile([128, 512], F32, tag="inv")
                    nc.vector.reciprocal(inv[64:128, :], av[64:128, :])
                    nc.vector.tensor_tensor(
                        out=ot_dst[h][:, ts(qq, 512)],
                        in0=av[0:64, :],
                        in1=inv[64:128, :],
                        op=mybir.AluOpType.mult,
                    )
            for mc in range(NDC):
                proj_mc(3, mc)

    nc.compile()
    return nc


@functools.lru_cache(maxsize=1)
def get_nc():
    return build_nc()


def make_in_maps(x, qkv_w, qkv_b, proj_w):
    """Per-core host-side sharding/layout prep."""
    x = np.asarray(x, dtype=np.float32)
    qkv_w = np.asarray(qkv_w, dtype=np.float32)
    qkv_b = np.asarray(qkv_b, dtype=np.float32)
    proj_w = np.asarray(proj_w, dtype=np.float32)

    in_maps = []
    for c in range(8):
        b, g = divmod(c, 4)
        h0, h1, h2 = 3 * g, 3 * g + 1, 3 * g + 2

        def qrows(h):
            return slice(h * HD, (h + 1) * HD)

        def krows(h):
            return slice(D + h * HD, D + (h + 1) * HD)

        def vrows(h):
            return slice(2 * D + h * HD, 2 * D + (h + 1) * HD)

        qk_order = [
            qrows(h0), qrows(h1), krows(h0), krows(h1), qrows(h2), krows(h2),
        ]
        wqk = np.concatenate([qkv_w[s] for s in qk_order], axis=0)   # (384, 768)
        bqk = np.concatenate([qkv_b[s] for s in qk_order], axis=0)   # (384,)
        bqkP = bqk.reshape(3, 128).T.copy()                          # (128, 3)
        wv = np.concatenate([qkv_w[vrows(h)] for h in (h0, h1, h2)], axis=0)
        bv = np.concatenate([qkv_b[vrows(h)] for h in (h0, h1, h2)], axis=0)
        wpp = np.concatenate(
            [proj_w[:, ts_np(h0)].T, proj_w[:, ts_np(h1)].T], axis=0
        )  # (128, 768)
        wps = np.concatenate(
            [proj_w[:, ts_np(h2)].T, np.zeros((64, D), np.float32)], axis=0
        )  # (128, 768)
        in_maps.append(
            {
                "xT": np.ascontiguousarray(x[b].reshape(L, D).T).astype(
                    ml_dtypes.bfloat16
                ),
                "wqkT": np.ascontiguousarray(wqk.T).astype(ml_dtypes.bfloat16),
                "wvT": np.ascontiguousarray(wv.T).astype(ml_dtypes.bfloat16),
                "bqkP": np.ascontiguousarray(bqkP),
                "bv1": bv.reshape(1, 192).astype(ml_dtypes.bfloat16),
                "ones": np.ones((1, 512), ml_dtypes.bfloat16),
                "wprojT": np.ascontiguousarray(
                    np.concatenate([wpp, wps], axis=1)
                ).astype(ml_dtypes.bfloat16),
            }
        )
    return in_maps


def ts_np(h):
    return slice(h * HD, (h + 1) * HD)


def assemble_output(results, proj_b):
    proj_b = np.asarray(proj_b, dtype=np.float32)
    out = np.zeros((B, L, D), np.float32)
    for c in range(8):
        b = c // 4
        out[b] += results[c]["zT"].T.astype(np.float32)
    out += proj_b[None, None, :]
    return out.reshape(B, T, N, D)


def _install_ntff_hook():
    """The container's antenv stub lacks axon_hooks; recreate it from the
    boot helper so trace=True can profile through libaxon_pjrt."""
    import sys
    import types

    try:
        from antenv.axon_hooks import get_axon_ntff_profile_hook  # noqa: F401

        return
    except ImportError:
        pass
    import antenv
    from trn_agent_boot.trn_boot import _ntff_profile_via_ctypes

    state = {"hook": _ntff_profile_via_ctypes("/opt/axon/libaxon_pjrt.so")}
    mod = types.ModuleType("antenv.axon_hooks")
    mod.set_axon_ntff_profile_hook = lambda h: state.__setitem__("hook", h)
    mod.get_axon_ntff_profile_hook = lambda: state["hook"]
    sys.modules["antenv.axon_hooks"] = mod
    antenv.axon_hooks = mod

    import concourse.bass_utils as bu

    orig_upload = bu.upload_artifacts

    def safe_upload(tmpdir):
        try:
            return orig_upload(tmpdir)
        except Exception:
            return tmpdir

    bu.upload_artifacts = safe_upload


def kernel_with_stats(x, qkv_w, qkv_b, proj_w, proj_b, trace=False):
    from concourse.bass_utils import run_bass_kernel_spmd

    if trace:
        _install_ntff_hook()
    nc = get_nc()
    in_maps = make_in_maps(x, qkv_w, qkv_b, proj_w)
    res = run_bass_kernel_spmd(nc, in_maps, list(range(8)), trace=trace)
    return assemble_output(res.results, proj_b), res


def kernel(x, qkv_w, qkv_b, proj_w, proj_b):
    out, _ = kernel_with_stats(x, qkv_w, qkv_b, proj_w, proj_b)
    return out
